# revision 31
# baseline (speedup 1.0000x reference)
"""HardAttentionMemoryAE Trainium2 kernel (v2: software-pipelined).

Data-parallel over 8 NeuronCores: x sharded along batch, weights + 50x128
memory bank replicated. Per core the pipeline runs in "transposed
activation" layout (features on partitions, rows on the free dim) so every
matmul contracts along partitions, with a row-major detour for the top-k
masking (per-row ops need rows on partitions).

v2 changes vs v1:
- Emission order software-pipelines slab s's encoder against slab s-1's
  attention/decoder tail so the in-order PE queue never idles (keeps the
  tensor engine p-state at max).
- Top-k thresholding runs on RAW sims (scale-invariant); 1/||z|| is folded
  into the Exp activation's per-partition scale operand.
- Row norms: z row-tiles are transposed on the PE, squared+row-reduced on
  DVE, and 1/sqrt computed with the int-bit-trick + 2 Newton steps on DVE
  (all [128,4] column-layout ops; no serial [1,512] work, no Sqrt table).
- Sigmoid replaced by 0.5*tanh(0.5x)+0.5: tanh/exp/relu/identity/copy all
  live in one activation table set -> zero steady-state ACT_TABLE_LOADs.
  The affine runs on the idle GpSimd(Pool) engine.
- PSUM: 5 rings x {2,2,2,1,1} banks so encoder/decoder/topk phases don't
  serialize on one bank.
"""
import numpy as np
import concourse.bass as bass
import concourse.mybir as mybir
from concourse import bacc
from concourse.tile import TileContext
from concourse.masks import make_identity
from concourse.bass_utils import run_bass_kernel_spmd

F32 = mybir.dt.float32
F32R = mybir.dt.float32r
I32 = mybir.dt.int32
AF = mybir.ActivationFunctionType
ALU = mybir.AluOpType

B_FULL = 65536
D = 784          # input dim
E = 128          # embed dim
M = 50           # memory slots
H = 256          # hidden
N_CORES = 8
SLAB = 512       # rows per slab (4 row-tiles of 128)
NHALF = 392      # final matmul N split (per PSUM bank, >=256 keeps f32r rate)

TRACE = False    # set by test harness for profiling runs

# engine assignment for the 7 per-slab xT PSUM->SBUF copies
XT_COPY_ENG = ["v", "v", "v", "v", "s", "v", "v"]
NEWTON_STEPS = 1


def _build(rows: int, n_cores: int, bias_mm: bool):
    nc = bacc.Bacc(
        "TRN2", target_bir_lowering=False, debug=False,
        enable_asserts=True, num_devices=n_cores
    )
    x = nc.dram_tensor("x", [rows, D], F32, kind="ExternalInput")
    W1 = nc.dram_tensor("W1", [D, H], F32, kind="ExternalInput")
    b1 = nc.dram_tensor("b1", [H], F32, kind="ExternalInput")
    W2 = nc.dram_tensor("W2", [H, E], F32, kind="ExternalInput")
    b2 = nc.dram_tensor("b2", [E], F32, kind="ExternalInput")
    mem = nc.dram_tensor("memory", [M, E], F32, kind="ExternalInput")
    W3 = nc.dram_tensor("W3", [E, H], F32, kind="ExternalInput")
    b3 = nc.dram_tensor("b3", [H], F32, kind="ExternalInput")
    W4 = nc.dram_tensor("W4", [H, D], F32, kind="ExternalInput")
    b4 = nc.dram_tensor("b4", [D], F32, kind="ExternalInput")
    y = nc.dram_tensor("y", [rows, D], F32, kind="ExternalOutput")

    n_slabs = rows // SLAB
    # x col chunks for the transpose: 6 aligned chunks + one overlapping
    # final chunk [656, 784) whose first 112 weight rows are zeroed.
    COFF = [0, 128, 256, 384, 512, 640, D - 128]

    x_r = x[:].rearrange("(s t p) c -> s p t c", p=128, t=4)
    y_r = y[:].rearrange("(s t p) c -> s p t c", p=128, t=4)

    with TileContext(nc) as tc:
        with (
            tc.tile_pool(name="const", bufs=1) as cpool,
            tc.tile_pool(name="xr", bufs=2) as xr_pool,
            tc.tile_pool(name="xT", bufs=2) as xT_pool,
            tc.tile_pool(name="hT", bufs=2) as hT_pool,
            tc.tile_pool(name="zT", bufs=2) as zT_pool,
            tc.tile_pool(name="small", bufs=2) as sm_pool,
            tc.tile_pool(name="xout", bufs=2) as xo_pool,
            tc.tile_pool(name="pbig", bufs=2, space="PSUM") as pbig,
            tc.tile_pool(name="pmid", bufs=3, space="PSUM") as pmid,
            tc.tile_pool(name="pxp", bufs=1, space="PSUM") as pxp,
            tc.tile_pool(name="pmisc", bufs=1, space="PSUM") as pmisc,
        ):
            st = {}   # slab index -> dict of live tiles

            def emit_dma_in(s):
                d = st.setdefault(s, {})
                d["xr"] = xr_pool.tile([128, 4, D], F32R, tag="xr",
                                       name=f"xr_{s}")
                nc.gpsimd.dma_start(d["xr"][:], x_r[s])

            # prefetch the first two x slabs before the weight DMAs so the
            # first transposes start as early as possible
            emit_dma_in(0)
            emit_dma_in(1)

            # ---------------- one-time setup ----------------
            W1sb = cpool.tile([128, 7 * H], F32R)
            zpad = cpool.tile([128, H], F32)
            nc.vector.memset(zpad[:], 0.0)
            nc.scalar.copy(W1sb[:, 6 * H:], zpad[:])
            for c in range(6):
                nc.gpsimd.dma_start(
                    W1sb[:, c * H:(c + 1) * H], W1[COFF[c]:COFF[c] + 128, :]
                )
            nc.gpsimd.dma_start(W1sb[112:128, 6 * H:7 * H], W1[768:D, :])
            W2sb = cpool.tile([128, 2 * E], F32R)
            for m in range(2):
                nc.gpsimd.dma_start(
                    W2sb[:, m * E:(m + 1) * E], W2[m * 128:(m + 1) * 128, :]
                )
            W3sb = cpool.tile([128, H], F32R)
            nc.gpsimd.dma_start(W3sb[:], W3[:])
            W4sb = cpool.tile([128, 2 * D], F32R)
            for k in range(2):
                nc.gpsimd.dma_start(
                    W4sb[:, k * D:(k + 1) * D], W4[k * 128:(k + 1) * 128, :]
                )
            b1sb = cpool.tile([128, 2], F32)
            nc.sync.dma_start(b1sb[:], b1[:].rearrange("(m p) -> p m", p=128))
            b2sb = cpool.tile([128, 1], F32)
            nc.sync.dma_start(b2sb[:], b2[:].rearrange("(p o) -> p o", o=1))
            b3sb = cpool.tile([128, 2], F32)
            nc.sync.dma_start(b3sb[:], b3[:].rearrange("(m p) -> p m", p=128))
            if bias_mm:
                b4row = cpool.tile([1, D], F32R)
                nc.gpsimd.dma_start(b4row[:], b4[:].rearrange("(o c) -> o c", o=1))
                ones_row = cpool.tile([1, 128], F32R)
                onesr_f = cpool.tile([1, 128], F32)
                nc.vector.memset(onesr_f[:], 1.0)
                nc.scalar.copy(ones_row[:], onesr_f[:])

            ident_f = cpool.tile([128, 128], F32)
            make_identity(nc, ident_f[:])
            ident = cpool.tile([128, 128], F32R)
            nc.scalar.copy(ident[:], ident_f[:])

            # normalized memory, transposed: mem_normT [E, M]
            memf = cpool.tile([M, E], F32)
            nc.sync.dma_start(memf[:], mem[:])
            msq = cpool.tile([M, E], F32)
            nc.scalar.square(msq[:], memf[:])
            mss = cpool.tile([M, 1], F32)
            nc.vector.tensor_reduce(mss[:], msq[:], mybir.AxisListType.X, ALU.add)
            nc.scalar.sqrt(mss[:], mss[:])
            nc.vector.tensor_scalar_max(mss[:], mss[:], 1e-12)
            minv = cpool.tile([M, 1], F32)
            nc.vector.reciprocal(minv[:], mss[:])
            mnorm = cpool.tile([M, E], F32R)
            nc.vector.tensor_scalar_mul(mnorm[:], memf[:], minv[:, 0:1])
            p_mn = pmisc.tile([128, 512], F32, tag="misc", name="p_mn")
            nc.tensor.transpose(p_mn[:E, :M].bitcast(F32R), mnorm[:], ident[:M, :M])
            mnT = cpool.tile([E, M], F32R)
            nc.scalar.copy(mnT[:], p_mn[:E, :M])

            # W3m = memory @ W3  [M, H]; decoder contracts attnT against it
            # directly (z_mem never materializes: attn@(mem@W3) == (attn@mem)@W3)
            p_mt = pmisc.tile([128, 512], F32, tag="misc", name="p_mt")
            memr = cpool.tile([M, E], F32R)
            nc.scalar.copy(memr[:], memf[:])
            nc.tensor.transpose(p_mt[:E, :M].bitcast(F32R), memr[:], ident[:M, :M])
            memT = cpool.tile([E, M], F32R)
            nc.scalar.copy(memT[:], p_mt[:E, :M])
            p_w3m = pmisc.tile([128, 512], F32, tag="misc", name="p_w3m")
            nc.tensor.matmul(p_w3m[:M, :H], memT[:], W3sb[:],
                             start=True, stop=True)
            W3msb = cpool.tile([M, H], F32R)
            nc.scalar.copy(W3msb[:], p_w3m[:M, :H])

            # ---------------- per-slab stage emitters ----------------
            def copy_eng(which):
                return {"v": nc.vector, "s": nc.scalar, "p": nc.gpsimd}[which]

            def emit_transp(s, chunks):
                d = st[s]
                xT = d.setdefault("xT", {})
                for c in chunks:
                    xT[c] = xT_pool.tile([128, SLAB], F32R, tag=f"xt{c}",
                                         name=f"xt{c}_{s}")
                    ptr = pmid.tile([128, 512], F32, tag="mid",
                                    name=f"ptr{c}_{s}")
                    for t in range(4):
                        nc.tensor.transpose(
                            ptr[:, t * 128:(t + 1) * 128].bitcast(F32R),
                            d["xr"][:, t, COFF[c]:COFF[c] + 128],
                            ident[:],
                        )
                    eng = copy_eng(XT_COPY_ENG[c])
                    if XT_COPY_ENG[c] == "s":
                        nc.scalar.copy(xT[c][:], ptr[:])
                    else:
                        eng.tensor_copy(xT[c][:], ptr[:])

            def emit_p1(s, m):
                d = st[s]
                if "hT" not in d:
                    d["hT"] = hT_pool.tile([128, 1024], F32R, tag="hT",
                                           name=f"hT_{s}")
                ph = pbig.tile([128, 512], F32, tag="big", name=f"ph{m}_{s}")
                for c in range(7):
                    nc.tensor.matmul(
                        ph[:],
                        W1sb[:, c * H + m * 128: c * H + m * 128 + 128],
                        d["xT"][c][:],
                        start=(c == 0), stop=(c == 6),
                    )
                nc.scalar.activation(
                    d["hT"][:, m * 512:(m + 1) * 512], ph[:],
                    AF.Relu, bias=b1sb[:, m:m + 1],
                )

            def emit_p2(s):
                d = st[s]
                pz = pmisc.tile([128, 512], F32, tag="misc", name=f"pz_{s}")
                for m in range(2):
                    nc.tensor.matmul(
                        pz[:], W2sb[:, m * E:(m + 1) * E],
                        d["hT"][:, m * 512:(m + 1) * 512],
                        start=(m == 0), stop=(m == 1),
                    )
                d["zT"] = zT_pool.tile([128, SLAB], F32R, tag="zT",
                                       name=f"zT_{s}")
                nc.scalar.activation(d["zT"][:], pz[:], AF.Identity,
                                     bias=b2sb[:, 0:1])

            def emit_norm(s):
                # row norms in column layout: transpose z row-tiles on PE,
                # square+reduce on DVE, rsqrt via bit trick + 2 Newton steps.
                d = st[s]
                zrm = pmisc.tile([128, 512], F32, tag="misc", name=f"zrm_{s}")
                for t in range(4):
                    nc.tensor.transpose(
                        zrm[:, t * 128:(t + 1) * 128].bitcast(F32R),
                        d["zT"][:, t * 128:(t + 1) * 128],
                        ident[:],
                    )
                zsqc = sm_pool.tile([128, 512], F32, tag="zsqc",
                                    name=f"zsqc_{s}")
                nc.scalar.square(zsqc[:], zrm[:])
                nsq = sm_pool.tile([128, 4], F32, tag="nsq", name=f"nsq_{s}")
                nc.vector.tensor_reduce(
                    nsq[:], zsqc[:].rearrange("p (t c) -> p t c", c=128),
                    mybir.AxisListType.X, ALU.add,
                )
                # inv = 1/sqrt(nsq): magic-constant seed + 2 Newton steps
                seed_i = sm_pool.tile([128, 4], I32, tag="seed_i",
                                      name=f"seed_i_{s}")
                nc.vector.tensor_scalar(
                    out=seed_i[:], in0=nsq[:].bitcast(I32),
                    scalar1=1, scalar2=None, op0=ALU.logical_shift_right,
                )
                y0_i = sm_pool.tile([128, 4], I32, tag="y0_i",
                                    name=f"y0_i_{s}")
                nc.vector.tensor_scalar(
                    out=y0_i[:], in0=seed_i[:],
                    scalar1=-1, scalar2=0x5F3759DF, op0=ALU.mult,
                    op1=ALU.add,
                )
                # y0_i = 0x5f3759df - (bits(nsq) >> 1): rsqrt seed
                h = sm_pool.tile([128, 4], F32, tag="h", name=f"h_{s}")
                nc.gpsimd.tensor_scalar(
                    out=h[:], in0=nsq[:], scalar1=0.5, scalar2=1e-30,
                    op0=ALU.mult, op1=ALU.max,
                )
                ycur = y0_i[:].bitcast(F32)
                for it in range(NEWTON_STEPS):
                    a = sm_pool.tile([128, 4], F32, tag=f"nta{it}",
                                     name=f"nta{it}_{s}")
                    nc.gpsimd.tensor_tensor(a[:], ycur, ycur, ALU.mult)
                    b_ = sm_pool.tile([128, 4], F32, tag=f"ntb{it}",
                                      name=f"ntb{it}_{s}")
                    nc.gpsimd.tensor_tensor(b_[:], a[:], h[:], ALU.mult)
                    c_ = sm_pool.tile([128, 4], F32, tag=f"ntc{it}",
                                      name=f"ntc{it}_{s}")
                    nc.gpsimd.tensor_scalar(
                        out=c_[:], in0=b_[:], scalar1=-1.0, scalar2=1.5,
                        op0=ALU.mult, op1=ALU.add,
                    )
                    ynext = sm_pool.tile([128, 4], F32, tag=f"nty{it}",
                                         name=f"nty{it}_{s}")
                    nc.gpsimd.tensor_tensor(ynext[:], ycur, c_[:], ALU.mult)
                    ycur = ynext[:]
                d["invcol"] = ycur

            def emit_sims(s):
                d = st[s]
                psim = pmisc.tile([128, 512], F32, tag="misc", name=f"psim_{s}")
                for t in range(4):
                    nc.tensor.matmul(
                        psim[:, t * M:(t + 1) * M],
                        d["zT"][:, t * 128:(t + 1) * 128], mnT[:],
                        start=True, stop=True,
                    )
                d["psim"] = psim

            def emit_topk(s):
                d = st[s]
                simsb = sm_pool.tile([128, 4 * M], F32, tag="simsb",
                                     name=f"simsb_{s}")
                nc.vector.tensor_copy(simsb[:], d["psim"][:, :4 * M])
                m8 = sm_pool.tile([128, 32], F32, tag="m8", name=f"m8_{s}")
                msk = sm_pool.tile([128, 4 * M], F32, tag="msk",
                                   name=f"msk_{s}")
                pexp = sm_pool.tile([128, 4 * M], F32, tag="pexp",
                                    name=f"pexp_{s}")
                den = sm_pool.tile([128, 4], F32, tag="den", name=f"den_{s}")
                for t in range(4):
                    nc.vector.max(m8[:, t * 8:(t + 1) * 8],
                                  simsb[:, t * M:(t + 1) * M])
                    nc.vector.scalar_tensor_tensor(
                        out=msk[:, t * M:(t + 1) * M],
                        in0=simsb[:, t * M:(t + 1) * M],
                        scalar=m8[:, t * 8 + 4:t * 8 + 5],
                        in1=simsb[:, t * M:(t + 1) * M],
                        op0=ALU.is_ge, op1=ALU.mult,
                    )
                    nc.scalar.activation(
                        pexp[:, t * M:(t + 1) * M], msk[:, t * M:(t + 1) * M],
                        AF.Exp, scale=d["invcol"][:, t:t + 1],
                        accum_out=den[:, t:t + 1],
                    )
                rden = sm_pool.tile([128, 4], F32, tag="rden",
                                    name=f"rden_{s}")
                nc.vector.reciprocal(rden[:], den[:])
                attn = sm_pool.tile([128, 4 * M], F32R, tag="attn",
                                    name=f"attn_{s}")
                for t in range(4):
                    nc.vector.tensor_scalar_mul(
                        attn[:, t * M:(t + 1) * M],
                        pexp[:, t * M:(t + 1) * M],
                        rden[:, t:t + 1],
                    )
                d["attn"] = attn

            def emit_pat(s):
                d = st[s]
                pat = pmisc.tile([128, 512], F32, tag="misc", name=f"pat_{s}")
                for t in range(4):
                    nc.tensor.transpose(
                        pat[:M, t * 128:(t + 1) * 128].bitcast(F32R),
                        d["attn"][:, t * M:(t + 1) * M], ident[:],
                    )
                attnT = sm_pool.tile([M, SLAB], F32R, tag="attnT",
                                     name=f"attnT_{s}")
                nc.scalar.copy(attnT[:], pat[:M, :])
                d["attnT"] = attnT

            def emit_dec(s):
                d = st[s]
                d["dT"] = hT_pool.tile([128, 1024], F32R, tag="dT",
                                       name=f"dT_{s}")
                for m in range(2):
                    pd = pbig.tile([128, 512], F32, tag="big",
                                   name=f"pd{m}_{s}")
                    nc.tensor.matmul(
                        pd[:], W3msb[:, m * 128:(m + 1) * 128], d["attnT"][:],
                        start=True, stop=True,
                    )
                    nc.scalar.activation(
                        d["dT"][:, m * 512:(m + 1) * 512], pd[:],
                        AF.Relu, bias=b3sb[:, m:m + 1],
                    )

            def emit_final(s, tiles):
                d = st[s]
                if "xo" not in d:
                    d["xo"] = xo_pool.tile([128, 4, D], F32, tag="xo",
                                           name=f"xo_{s}")
                for t in tiles:
                    px = pxp.tile([128, 1024], F32, tag="x",
                                  name=f"px{t}_{s}")
                    for nh in range(2):
                        pxh = px[:, nh * 512:nh * 512 + NHALF]
                        if bias_mm:
                            nc.tensor.matmul(
                                pxh, ones_row[:],
                                b4row[:, nh * NHALF:(nh + 1) * NHALF],
                                start=True, stop=False,
                            )
                        for k in range(2):
                            nc.tensor.matmul(
                                pxh,
                                d["dT"][:, k * 512 + t * 128:
                                        k * 512 + t * 128 + 128],
                                W4sb[:, k * D + nh * NHALF:
                                     k * D + (nh + 1) * NHALF],
                                start=(k == 0 and not bias_mm), stop=(k == 1),
                            )
                    # sigmoid(v) = 0.5*tanh(0.5 v)+0.5; one 3D-AP tanh per
                    # row tile, affine fixup on the (otherwise idle) pool
                    pxv = px[:].rearrange("p (n c) -> p n c", n=2)[:, :, :NHALF]
                    xov = d["xo"][:, t, :].rearrange("p (n c) -> p n c", n=2)
                    nc.scalar.activation(xov, pxv, AF.Tanh, scale=0.5)
                    nc.gpsimd.tensor_scalar(
                        out=d["xo"][:, t, :], in0=d["xo"][:, t, :],
                        scalar1=0.5, scalar2=0.5,
                        op0=ALU.mult, op1=ALU.add,
                    )

            def emit_out(s):
                nc.sync.dma_start(y_r[s], st[s]["xo"][:])
                # drop references to this slab's tiles
                del st[s]

            # ---------------- software-pipelined slab loop ----------------
            for s in range(n_slabs):
                emit_transp(s, [0, 1, 2, 3, 4, 5, 6])
                emit_p1(s, 0)
                if s > 0:
                    emit_topk(s - 1)
                emit_p1(s, 1)
                if s > 0:
                    emit_pat(s - 1)
                    emit_dec(s - 1)
                    emit_final(s - 1, [0, 1])
                emit_p2(s)
                emit_norm(s)
                emit_sims(s)
                if s + 2 < n_slabs:
                    emit_dma_in(s + 2)
                if s > 0:
                    emit_final(s - 1, [2, 3])
                    emit_out(s - 1)
            s = n_slabs - 1
            emit_topk(s)
            emit_pat(s)
            emit_dec(s)
            emit_final(s, [0, 1, 2, 3])
            emit_out(s)

    nc.finalize()
    return nc


_cache: dict = {}


def _get_nc(rows: int, n_cores: int, bias_mm: bool):
    key = (rows, n_cores, bias_mm)
    if key not in _cache:
        _cache[key] = _build(rows, n_cores, bias_mm)
    return _cache[key]


def kernel(**inputs):
    x = np.ascontiguousarray(np.asarray(inputs["x"], dtype=np.float32))
    rows = x.shape[0]
    n_cores = N_CORES
    rows_pc = rows // n_cores
    bias_mm = not np.allclose(np.asarray(inputs["b4"]), 0.0)
    nc = _get_nc(rows_pc, n_cores, bias_mm)

    w_keys = ["W1", "b1", "W2", "b2", "memory", "W3", "b3", "W4", "b4"]
    weights = {
        k: np.ascontiguousarray(np.asarray(inputs[k], dtype=np.float32))
        for k in w_keys
    }
    in_maps = [
        {"x": x[c * rows_pc:(c + 1) * rows_pc], **weights}
        for c in range(n_cores)
    ]
    res = run_bass_kernel_spmd(
        nc, in_maps, list(range(n_cores)), trace=TRACE
    )
    kernel.last_result = res
    y = np.concatenate([res.results[c]["y"] for c in range(n_cores)], axis=0)
    return y.astype(np.float32)


# revision 32
# speedup vs baseline: 1.0042x; 1.0042x over previous
"""HardAttentionMemoryAE Trainium2 kernel (software-pipelined).

Data-parallel over 8 NeuronCores: x sharded along batch, weights + 50x128
memory bank replicated. Per core the pipeline runs in "transposed
activation" layout (features on partitions, rows on the free dim) so every
matmul contracts along partitions, with a row-major detour for the top-k
masking (per-row ops need rows on partitions).

Optimizations vs the naive slab loop (483us -> ~315us):
- Emission order software-pipelines slab s's encoder against slab s-1's
  attention/decoder tail so the in-order PE queue rarely idles; dense PE
  occupancy keeps the tensor engine at its max p-state (2.4GHz vs 1.2).
- Top-k thresholding runs on raw sims (scale-invariant); 1/||z|| is folded
  into the Exp activation's per-partition scale operand.
- Row norms: z row-tiles transposed on the PE, squared+row-reduced on DVE,
  1/sqrt via int-bit-trick seed + Newton on GpSimd ([128,4] column-layout
  ops only; no serial [1,512] work, no Sqrt table load).
- Sigmoid == 0.5*tanh(0.5x)+0.5: tanh/exp/relu/identity/square/copy all
  live in one activation-table set -> zero steady-state ACT_TABLE_LOADs.
  The output affine runs on the otherwise idle GpSimd engine.
- W3m = memory @ W3 precomputed once; the decoder contracts attnT against
  it directly, so z_mem (matmul + PSUM->SBUF copy) never materializes.
- PSUM: 4 rings x {2,3,2,1} banks; the misc ring serializes pat->pz->
  zrm->psim which matches their natural dataflow order.
- PSUM->SBUF copies split across DVE/scalar so neither queue backlogs the
  transposes that feed phase 1.
"""
import numpy as np
import concourse.bass as bass
import concourse.mybir as mybir
from concourse import bacc
from concourse.tile import TileContext
from concourse.masks import make_identity
from concourse.bass_utils import run_bass_kernel_spmd

F32 = mybir.dt.float32
F32R = mybir.dt.float32r
I32 = mybir.dt.int32
AF = mybir.ActivationFunctionType
ALU = mybir.AluOpType

B_FULL = 65536
D = 784          # input dim
E = 128          # embed dim
M = 50           # memory slots
H = 256          # hidden
N_CORES = 8
SLAB = 512       # rows per slab (4 row-tiles of 128)
NHALF = 392      # final matmul N split (per PSUM bank, >=256 keeps f32r rate)

TRACE = False    # set by test harness for profiling runs

# engine assignment for the 7 per-slab xT PSUM->SBUF copies
XT_COPY_ENG = ["v", "v", "v", "v", "s", "v", "v"]
NEWTON_STEPS = 1


def _build(rows: int, n_cores: int, bias_mm: bool):
    nc = bacc.Bacc(
        "TRN2", target_bir_lowering=False, debug=False,
        enable_asserts=True, num_devices=n_cores
    )
    x = nc.dram_tensor("x", [rows, D], F32, kind="ExternalInput")
    W1 = nc.dram_tensor("W1", [D, H], F32, kind="ExternalInput")
    b1 = nc.dram_tensor("b1", [H], F32, kind="ExternalInput")
    W2 = nc.dram_tensor("W2", [H, E], F32, kind="ExternalInput")
    b2 = nc.dram_tensor("b2", [E], F32, kind="ExternalInput")
    mem = nc.dram_tensor("memory", [M, E], F32, kind="ExternalInput")
    W3 = nc.dram_tensor("W3", [E, H], F32, kind="ExternalInput")
    b3 = nc.dram_tensor("b3", [H], F32, kind="ExternalInput")
    W4 = nc.dram_tensor("W4", [H, D], F32, kind="ExternalInput")
    b4 = nc.dram_tensor("b4", [D], F32, kind="ExternalInput")
    y = nc.dram_tensor("y", [rows, D], F32, kind="ExternalOutput")

    n_slabs = rows // SLAB
    # x col chunks for the transpose: 6 aligned chunks + one overlapping
    # final chunk [656, 784) whose first 112 weight rows are zeroed.
    COFF = [0, 128, 256, 384, 512, 640, D - 128]

    x_r = x[:].rearrange("(s t p) c -> s p t c", p=128, t=4)
    y_r = y[:].rearrange("(s t p) c -> s p t c", p=128, t=4)

    with TileContext(nc) as tc:
        with (
            tc.tile_pool(name="const", bufs=1) as cpool,
            tc.tile_pool(name="xr", bufs=2) as xr_pool,
            tc.tile_pool(name="xT", bufs=2) as xT_pool,
            tc.tile_pool(name="hT", bufs=2) as hT_pool,
            tc.tile_pool(name="zT", bufs=2) as zT_pool,
            tc.tile_pool(name="small", bufs=2) as sm_pool,
            tc.tile_pool(name="xout", bufs=2) as xo_pool,
            tc.tile_pool(name="pbig", bufs=2, space="PSUM") as pbig,
            tc.tile_pool(name="pmid", bufs=3, space="PSUM") as pmid,
            tc.tile_pool(name="pxp", bufs=1, space="PSUM") as pxp,
            tc.tile_pool(name="pmisc", bufs=1, space="PSUM") as pmisc,
        ):
            st = {}   # slab index -> dict of live tiles

            def emit_dma_in(s):
                d = st.setdefault(s, {})
                d["xr"] = xr_pool.tile([128, 4, D], F32R, tag="xr",
                                       name=f"xr_{s}")
                nc.gpsimd.dma_start(d["xr"][:], x_r[s])

            # prefetch the first two x slabs before the weight DMAs so the
            # first transposes start as early as possible
            emit_dma_in(0)
            emit_dma_in(1)

            # ---------------- one-time setup ----------------
            W1sb = cpool.tile([128, 7 * H], F32R)
            zpad = cpool.tile([128, H], F32)
            nc.vector.memset(zpad[:], 0.0)
            nc.scalar.copy(W1sb[:, 6 * H:], zpad[:])
            for c in range(6):
                nc.gpsimd.dma_start(
                    W1sb[:, c * H:(c + 1) * H], W1[COFF[c]:COFF[c] + 128, :]
                )
            nc.gpsimd.dma_start(W1sb[112:128, 6 * H:7 * H], W1[768:D, :])
            W2sb = cpool.tile([128, 2 * E], F32R)
            for m in range(2):
                nc.gpsimd.dma_start(
                    W2sb[:, m * E:(m + 1) * E], W2[m * 128:(m + 1) * 128, :]
                )
            W3sb = cpool.tile([128, H], F32R)
            nc.gpsimd.dma_start(W3sb[:], W3[:])
            W4sb = cpool.tile([128, 2 * D], F32R)
            for k in range(2):
                nc.gpsimd.dma_start(
                    W4sb[:, k * D:(k + 1) * D], W4[k * 128:(k + 1) * 128, :]
                )
            b1sb = cpool.tile([128, 2], F32)
            nc.sync.dma_start(b1sb[:], b1[:].rearrange("(m p) -> p m", p=128))
            b2sb = cpool.tile([128, 1], F32)
            nc.sync.dma_start(b2sb[:], b2[:].rearrange("(p o) -> p o", o=1))
            b3sb = cpool.tile([128, 2], F32)
            nc.sync.dma_start(b3sb[:], b3[:].rearrange("(m p) -> p m", p=128))
            if bias_mm:
                b4row = cpool.tile([1, D], F32R)
                nc.gpsimd.dma_start(b4row[:], b4[:].rearrange("(o c) -> o c", o=1))
                ones_row = cpool.tile([1, 128], F32R)
                onesr_f = cpool.tile([1, 128], F32)
                nc.vector.memset(onesr_f[:], 1.0)
                nc.scalar.copy(ones_row[:], onesr_f[:])

            ident_f = cpool.tile([128, 128], F32)
            make_identity(nc, ident_f[:])
            ident = cpool.tile([128, 128], F32R)
            nc.scalar.copy(ident[:], ident_f[:])

            # normalized memory, transposed: mem_normT [E, M]
            memf = cpool.tile([M, E], F32)
            nc.sync.dma_start(memf[:], mem[:])
            msq = cpool.tile([M, E], F32)
            nc.scalar.square(msq[:], memf[:])
            mss = cpool.tile([M, 1], F32)
            nc.vector.tensor_reduce(mss[:], msq[:], mybir.AxisListType.X, ALU.add)
            nc.scalar.sqrt(mss[:], mss[:])
            nc.vector.tensor_scalar_max(mss[:], mss[:], 1e-12)
            minv = cpool.tile([M, 1], F32)
            nc.vector.reciprocal(minv[:], mss[:])
            mnorm = cpool.tile([M, E], F32R)
            nc.vector.tensor_scalar_mul(mnorm[:], memf[:], minv[:, 0:1])
            p_mn = pmisc.tile([128, 512], F32, tag="misc", name="p_mn")
            nc.tensor.transpose(p_mn[:E, :M].bitcast(F32R), mnorm[:], ident[:M, :M])
            mnT = cpool.tile([E, M], F32R)
            nc.scalar.copy(mnT[:], p_mn[:E, :M])

            # W3m = memory @ W3  [M, H]; decoder contracts attnT against it
            # directly (z_mem never materializes: attn@(mem@W3) == (attn@mem)@W3)
            p_mt = pmisc.tile([128, 512], F32, tag="misc", name="p_mt")
            memr = cpool.tile([M, E], F32R)
            nc.scalar.copy(memr[:], memf[:])
            nc.tensor.transpose(p_mt[:E, :M].bitcast(F32R), memr[:], ident[:M, :M])
            memT = cpool.tile([E, M], F32R)
            nc.scalar.copy(memT[:], p_mt[:E, :M])
            p_w3m = pmisc.tile([128, 512], F32, tag="misc", name="p_w3m")
            nc.tensor.matmul(p_w3m[:M, :H], memT[:], W3sb[:],
                             start=True, stop=True)
            W3msb = cpool.tile([M, H], F32R)
            nc.scalar.copy(W3msb[:], p_w3m[:M, :H])

            # ---------------- per-slab stage emitters ----------------
            def copy_eng(which):
                return {"v": nc.vector, "s": nc.scalar, "p": nc.gpsimd}[which]

            def emit_transp(s, chunks):
                d = st[s]
                xT = d.setdefault("xT", {})
                for c in chunks:
                    xT[c] = xT_pool.tile([128, SLAB], F32R, tag=f"xt{c}",
                                         name=f"xt{c}_{s}")
                    ptr = pmid.tile([128, 512], F32, tag="mid",
                                    name=f"ptr{c}_{s}")
                    for t in range(4):
                        nc.tensor.transpose(
                            ptr[:, t * 128:(t + 1) * 128].bitcast(F32R),
                            d["xr"][:, t, COFF[c]:COFF[c] + 128],
                            ident[:],
                        )
                    eng = copy_eng(XT_COPY_ENG[c])
                    if XT_COPY_ENG[c] == "s":
                        nc.scalar.copy(xT[c][:], ptr[:])
                    else:
                        eng.tensor_copy(xT[c][:], ptr[:])

            def emit_p1(s, m):
                d = st[s]
                if "hT" not in d:
                    d["hT"] = hT_pool.tile([128, 1024], F32R, tag="hT",
                                           name=f"hT_{s}")
                ph = pbig.tile([128, 512], F32, tag="big", name=f"ph{m}_{s}")
                for c in range(7):
                    nc.tensor.matmul(
                        ph[:],
                        W1sb[:, c * H + m * 128: c * H + m * 128 + 128],
                        d["xT"][c][:],
                        start=(c == 0), stop=(c == 6),
                    )
                nc.scalar.activation(
                    d["hT"][:, m * 512:(m + 1) * 512], ph[:],
                    AF.Relu, bias=b1sb[:, m:m + 1],
                )

            def emit_p2(s):
                d = st[s]
                pz = pmisc.tile([128, 512], F32, tag="misc", name=f"pz_{s}")
                for m in range(2):
                    nc.tensor.matmul(
                        pz[:], W2sb[:, m * E:(m + 1) * E],
                        d["hT"][:, m * 512:(m + 1) * 512],
                        start=(m == 0), stop=(m == 1),
                    )
                d["zT"] = zT_pool.tile([128, SLAB], F32R, tag="zT",
                                       name=f"zT_{s}")
                nc.scalar.activation(d["zT"][:], pz[:], AF.Identity,
                                     bias=b2sb[:, 0:1])

            def emit_norm(s):
                # row norms in column layout: transpose z row-tiles on PE,
                # square+reduce on DVE, rsqrt via bit trick + 2 Newton steps.
                d = st[s]
                zrm = pmisc.tile([128, 512], F32, tag="misc", name=f"zrm_{s}")
                for t in range(4):
                    nc.tensor.transpose(
                        zrm[:, t * 128:(t + 1) * 128].bitcast(F32R),
                        d["zT"][:, t * 128:(t + 1) * 128],
                        ident[:],
                    )
                zsqc = sm_pool.tile([128, 512], F32, tag="zsqc",
                                    name=f"zsqc_{s}")
                nc.scalar.square(zsqc[:], zrm[:])
                nsq = sm_pool.tile([128, 4], F32, tag="nsq", name=f"nsq_{s}")
                nc.vector.tensor_reduce(
                    nsq[:], zsqc[:].rearrange("p (t c) -> p t c", c=128),
                    mybir.AxisListType.X, ALU.add,
                )
                # inv = 1/sqrt(nsq): magic-constant seed + 2 Newton steps
                seed_i = sm_pool.tile([128, 4], I32, tag="seed_i",
                                      name=f"seed_i_{s}")
                nc.vector.tensor_scalar(
                    out=seed_i[:], in0=nsq[:].bitcast(I32),
                    scalar1=1, scalar2=None, op0=ALU.logical_shift_right,
                )
                y0_i = sm_pool.tile([128, 4], I32, tag="y0_i",
                                    name=f"y0_i_{s}")
                nc.vector.tensor_scalar(
                    out=y0_i[:], in0=seed_i[:],
                    scalar1=-1, scalar2=0x5F3759DF, op0=ALU.mult,
                    op1=ALU.add,
                )
                # y0_i = 0x5f3759df - (bits(nsq) >> 1): rsqrt seed
                h = sm_pool.tile([128, 4], F32, tag="h", name=f"h_{s}")
                nc.gpsimd.tensor_scalar(
                    out=h[:], in0=nsq[:], scalar1=0.5, scalar2=1e-30,
                    op0=ALU.mult, op1=ALU.max,
                )
                ycur = y0_i[:].bitcast(F32)
                for it in range(NEWTON_STEPS):
                    a = sm_pool.tile([128, 4], F32, tag=f"nta{it}",
                                     name=f"nta{it}_{s}")
                    nc.gpsimd.tensor_tensor(a[:], ycur, ycur, ALU.mult)
                    b_ = sm_pool.tile([128, 4], F32, tag=f"ntb{it}",
                                      name=f"ntb{it}_{s}")
                    nc.gpsimd.tensor_tensor(b_[:], a[:], h[:], ALU.mult)
                    c_ = sm_pool.tile([128, 4], F32, tag=f"ntc{it}",
                                      name=f"ntc{it}_{s}")
                    nc.gpsimd.tensor_scalar(
                        out=c_[:], in0=b_[:], scalar1=-1.0, scalar2=1.5,
                        op0=ALU.mult, op1=ALU.add,
                    )
                    ynext = sm_pool.tile([128, 4], F32, tag=f"nty{it}",
                                         name=f"nty{it}_{s}")
                    nc.gpsimd.tensor_tensor(ynext[:], ycur, c_[:], ALU.mult)
                    ycur = ynext[:]
                d["invcol"] = ycur

            def emit_sims(s):
                d = st[s]
                psim = pmisc.tile([128, 512], F32, tag="misc", name=f"psim_{s}")
                for t in range(4):
                    nc.tensor.matmul(
                        psim[:, t * M:(t + 1) * M],
                        d["zT"][:, t * 128:(t + 1) * 128], mnT[:],
                        start=True, stop=True,
                    )
                d["psim"] = psim

            def emit_topk(s):
                d = st[s]
                simsb = sm_pool.tile([128, 4 * M], F32, tag="simsb",
                                     name=f"simsb_{s}")
                nc.vector.tensor_copy(simsb[:], d["psim"][:, :4 * M])
                m8 = sm_pool.tile([128, 32], F32, tag="m8", name=f"m8_{s}")
                msk = sm_pool.tile([128, 4 * M], F32, tag="msk",
                                   name=f"msk_{s}")
                pexp = sm_pool.tile([128, 4 * M], F32, tag="pexp",
                                    name=f"pexp_{s}")
                den = sm_pool.tile([128, 4], F32, tag="den", name=f"den_{s}")
                for t in range(4):
                    nc.vector.max(m8[:, t * 8:(t + 1) * 8],
                                  simsb[:, t * M:(t + 1) * M])
                    nc.vector.scalar_tensor_tensor(
                        out=msk[:, t * M:(t + 1) * M],
                        in0=simsb[:, t * M:(t + 1) * M],
                        scalar=m8[:, t * 8 + 4:t * 8 + 5],
                        in1=simsb[:, t * M:(t + 1) * M],
                        op0=ALU.is_ge, op1=ALU.mult,
                    )
                    nc.scalar.activation(
                        pexp[:, t * M:(t + 1) * M], msk[:, t * M:(t + 1) * M],
                        AF.Exp, scale=d["invcol"][:, t:t + 1],
                        accum_out=den[:, t:t + 1],
                    )
                rden = sm_pool.tile([128, 4], F32, tag="rden",
                                    name=f"rden_{s}")
                nc.vector.reciprocal(rden[:], den[:])
                attn = sm_pool.tile([128, 4 * M], F32R, tag="attn",
                                    name=f"attn_{s}")
                for t in range(4):
                    nc.vector.tensor_scalar_mul(
                        attn[:, t * M:(t + 1) * M],
                        pexp[:, t * M:(t + 1) * M],
                        rden[:, t:t + 1],
                    )
                d["attn"] = attn

            def emit_pat(s):
                d = st[s]
                pat = pmisc.tile([128, 512], F32, tag="misc", name=f"pat_{s}")
                for t in range(4):
                    nc.tensor.transpose(
                        pat[:M, t * 128:(t + 1) * 128].bitcast(F32R),
                        d["attn"][:, t * M:(t + 1) * M], ident[:],
                    )
                attnT = sm_pool.tile([M, SLAB], F32R, tag="attnT",
                                     name=f"attnT_{s}")
                nc.vector.tensor_copy(attnT[:], pat[:M, :])
                d["attnT"] = attnT

            def emit_dec(s):
                d = st[s]
                d["dT"] = hT_pool.tile([128, 1024], F32R, tag="dT",
                                       name=f"dT_{s}")
                for m in range(2):
                    pd = pbig.tile([128, 512], F32, tag="big",
                                   name=f"pd{m}_{s}")
                    nc.tensor.matmul(
                        pd[:], W3msb[:, m * 128:(m + 1) * 128], d["attnT"][:],
                        start=True, stop=True,
                    )
                    nc.scalar.activation(
                        d["dT"][:, m * 512:(m + 1) * 512], pd[:],
                        AF.Relu, bias=b3sb[:, m:m + 1],
                    )

            def emit_final(s, tiles):
                d = st[s]
                if "xo" not in d:
                    d["xo"] = xo_pool.tile([128, 4, D], F32, tag="xo",
                                           name=f"xo_{s}")
                for t in tiles:
                    px = pxp.tile([128, 1024], F32, tag="x",
                                  name=f"px{t}_{s}")
                    for nh in range(2):
                        pxh = px[:, nh * 512:nh * 512 + NHALF]
                        if bias_mm:
                            nc.tensor.matmul(
                                pxh, ones_row[:],
                                b4row[:, nh * NHALF:(nh + 1) * NHALF],
                                start=True, stop=False,
                            )
                        for k in range(2):
                            nc.tensor.matmul(
                                pxh,
                                d["dT"][:, k * 512 + t * 128:
                                        k * 512 + t * 128 + 128],
                                W4sb[:, k * D + nh * NHALF:
                                     k * D + (nh + 1) * NHALF],
                                start=(k == 0 and not bias_mm), stop=(k == 1),
                            )
                    # sigmoid(v) = 0.5*tanh(0.5 v)+0.5; one 3D-AP tanh per
                    # row tile, affine fixup on the (otherwise idle) pool
                    pxv = px[:].rearrange("p (n c) -> p n c", n=2)[:, :, :NHALF]
                    xov = d["xo"][:, t, :].rearrange("p (n c) -> p n c", n=2)
                    nc.scalar.activation(xov, pxv, AF.Tanh, scale=0.5)
                    nc.gpsimd.tensor_scalar(
                        out=d["xo"][:, t, :], in0=d["xo"][:, t, :],
                        scalar1=0.5, scalar2=0.5,
                        op0=ALU.mult, op1=ALU.add,
                    )

            def emit_out(s):
                nc.sync.dma_start(y_r[s], st[s]["xo"][:])
                # drop references to this slab's tiles
                del st[s]

            # ---------------- software-pipelined slab loop ----------------
            for s in range(n_slabs):
                emit_transp(s, [0, 1, 2, 3, 4, 5, 6])
                emit_p1(s, 0)
                if s > 0:
                    emit_topk(s - 1)
                emit_p1(s, 1)
                if s > 0:
                    emit_pat(s - 1)
                    emit_dec(s - 1)
                    emit_final(s - 1, [0, 1])
                emit_p2(s)
                emit_norm(s)
                emit_sims(s)
                if s + 2 < n_slabs:
                    emit_dma_in(s + 2)
                if s > 0:
                    emit_final(s - 1, [2, 3])
                    emit_out(s - 1)
            s = n_slabs - 1
            emit_topk(s)
            emit_pat(s)
            emit_dec(s)
            emit_final(s, [0, 1, 2, 3])
            emit_out(s)

    nc.finalize()
    return nc


_cache: dict = {}


def _get_nc(rows: int, n_cores: int, bias_mm: bool):
    key = (rows, n_cores, bias_mm)
    if key not in _cache:
        _cache[key] = _build(rows, n_cores, bias_mm)
    return _cache[key]


def kernel(**inputs):
    x = np.ascontiguousarray(np.asarray(inputs["x"], dtype=np.float32))
    rows = x.shape[0]
    n_cores = N_CORES
    rows_pc = rows // n_cores
    bias_mm = not np.allclose(np.asarray(inputs["b4"]), 0.0)
    nc = _get_nc(rows_pc, n_cores, bias_mm)

    w_keys = ["W1", "b1", "W2", "b2", "memory", "W3", "b3", "W4", "b4"]
    weights = {
        k: np.ascontiguousarray(np.asarray(inputs[k], dtype=np.float32))
        for k in w_keys
    }
    in_maps = [
        {"x": x[c * rows_pc:(c + 1) * rows_pc], **weights}
        for c in range(n_cores)
    ]
    res = run_bass_kernel_spmd(
        nc, in_maps, list(range(n_cores)), trace=TRACE
    )
    kernel.last_result = res
    y = np.concatenate([res.results[c]["y"] for c in range(n_cores)], axis=0)
    return y.astype(np.float32)
